# revision 1
# baseline (speedup 1.0000x reference)
"""Trainium2 Bass kernel for nn_BMSampling: out = X.reshape(B*C, T) @ smp_weight.

Strategy:
- smp_weight columns are interpolation stencils; ~55.6% are entirely zero,
  so their output columns are exactly 0.0. The kernel compacts to nonzero
  columns at runtime (generic for any weight), computes only those on
  device, and scatters into a zero-filled full output on the host.
- Tensor-parallel over the compacted output columns: 8 cores x NSH each.
  Each core computes OUT_shard[512, NSH] = X^T[100,512].T @ W_shard[100,NSH].
- The kernel is output-DMA bound. To keep the PE off the critical path, the
  fp32 matmul (1/4 bf16 rate on trn2: 2 passes x 2 cycles/col) is replaced
  by a 3-term split-fp16 matmul: X = Xh+Xl, W = Wh+Wl (hi/lo fp16 pairs
  built on host), OUT = Xh@Wh + Xl@Wh + Xh@Wl accumulated in fp32 PSUM.
  Dropped Xl@Wl term is ~2^-22; end-to-end error ~1e-7 of scale. DMA bytes
  unchanged (two fp16 halves = one fp32).
- W loads ride the ACT HWDGE ring, output stores the SP ring (no
  head-of-line blocking); the first chunk streams in as 500-col strips so
  the PE starts ~2us in.
"""

from contextlib import ExitStack

import numpy as np

import concourse.bacc as bacc
import concourse.mybir as mybir
import concourse.tile as tile
from concourse import bass_utils

B, C, T = 4, 128, 100
N_SMP, D_PROP = 32, 100
M = B * C                     # 512 matmul rows
NDT = N_SMP * D_PROP * T      # 320000 output columns
NCORES = 8
GRANULE = 1000 * NCORES       # compact col count padded to this

K = T                         # 100 contraction dim (on SBUF partitions)
N_OUTER = 4000                # columns per W tile / output staging tile
N_INNER = 500                 # matmul free dim (fits one PSUM bank: <=512 f32)
F32 = mybir.dt.float32
F16 = mybir.dt.float16

_PROGRAMS = {}


def _build(nsh):
    """Per-core program computing OUT[512, nsh] = XT.T @ W[100, nsh]."""
    if nsh in _PROGRAMS:
        return _PROGRAMS[nsh]

    widths = [N_OUTER] * (nsh // N_OUTER)
    if nsh % N_OUTER:
        widths.append(nsh % N_OUTER)
    assert all(w % (2 * N_INNER) == 0 for w in widths), widths

    nc = bacc.Bacc("TRN2", debug=False)
    xhl = nc.dram_tensor("XHL", [2, K, M], F16, kind="ExternalInput").ap()
    whl = nc.dram_tensor("WHL", [2, K, nsh], F16, kind="ExternalInput").ap()
    out = nc.dram_tensor("OUT", [M, nsh], F32, kind="ExternalOutput").ap()

    with tile.TileContext(nc) as tc, ExitStack() as ctx:
        xpool = ctx.enter_context(tc.tile_pool(name="x", bufs=1))
        wpool = ctx.enter_context(tc.tile_pool(name="w", bufs=4))
        w0pool = ctx.enter_context(tc.tile_pool(name="w0", bufs=8))
        opool = ctx.enter_context(tc.tile_pool(name="o", bufs=4))
        pspool = ctx.enter_context(tc.tile_pool(name="ps", bufs=4, space="PSUM"))

        x_sb = xpool.tile([K, 2, M], F16)
        nc.scalar.dma_start(out=x_sb[:], in_=xhl.rearrange("a k m -> k a m"))
        xh_sb = x_sb[:, 0]
        xl_sb = x_sb[:, 1]

        n0 = 0
        for it, width in enumerate(widths):
            nj = width // N_INNER
            if it == 0:
                # First chunk streams in as 500-col strips so the first
                # matmul starts ~2us in instead of waiting on a 1.6 MB load.
                w_strips = []
                for j in range(nj):
                    sl = slice(n0 + j * N_INNER, n0 + (j + 1) * N_INNER)
                    ws = w0pool.tile([K, 2, N_INNER], F16, tag="w0")
                    nc.scalar.dma_start(
                        out=ws[:], in_=whl[:, :, sl].rearrange("a k n -> k a n")
                    )
                    w_strips.append(ws)
                strip = lambda j: (w_strips[j][:, 0], w_strips[j][:, 1])
            else:
                w_sb = wpool.tile([K, 2, N_OUTER], F16, tag="w_sb")
                nc.scalar.dma_start(
                    out=w_sb[:, :, :width],
                    in_=whl[:, :, n0 : n0 + width].rearrange("a k n -> k a n"),
                )
                strip = lambda j, a=w_sb: (
                    a[:, 0, j * N_INNER : (j + 1) * N_INNER],
                    a[:, 1, j * N_INNER : (j + 1) * N_INNER],
                )
            for m in range(M // 128):
                msl = slice(m * 128, (m + 1) * 128)
                o_sb = opool.tile([128, N_OUTER], F32, tag="o_sb")
                for j in range(0, nj, 2):
                    ps = pspool.tile([128, 2, 512], F32)  # one PSUM bank per slot
                    for h in range(2):
                        wsh, wsl = strip(j + h)
                        dst = ps[:, h, :N_INNER]
                        nc.tensor.matmul(
                            dst, xh_sb[:, msl], wsh, start=True, stop=False
                        )
                        nc.tensor.matmul(
                            dst, xl_sb[:, msl], wsh, start=False, stop=False
                        )
                        nc.tensor.matmul(
                            dst, xh_sb[:, msl], wsl, start=False, stop=True
                        )
                    nc.vector.tensor_copy(
                        out=o_sb[
                            :, j * N_INNER : (j + 2) * N_INNER
                        ].rearrange("p (a b) -> p a b", a=2),
                        in_=ps[:, :, :N_INNER],
                    )
                nc.sync.dma_start(
                    out=out[msl, n0 : n0 + width],
                    in_=o_sb[:, :width],
                )
            n0 += width

    nc.compile()
    _PROGRAMS[nsh] = nc
    return nc


def _split16(a):
    hi = a.astype(np.float16)
    lo = (a - hi.astype(np.float32)).astype(np.float16)
    return np.ascontiguousarray(hi), np.ascontiguousarray(lo)


def prepare_run(X, smp_weight):
    """Returns (nc, in_maps, assemble) where assemble(results)->full output."""
    X = np.ascontiguousarray(np.asarray(X, dtype=np.float32))
    Wfull = np.asarray(smp_weight, dtype=np.float32)

    # Compact away all-zero weight columns: their outputs are exactly 0.0.
    nz = np.flatnonzero((Wfull != 0).any(axis=0))
    padded = max(GRANULE, (len(nz) + GRANULE - 1) // GRANULE * GRANULE)
    nsh = padded // NCORES
    Wc = np.zeros((K, padded), dtype=np.float32)
    Wc[:, : len(nz)] = Wfull[:, nz]

    xt = np.ascontiguousarray(X.reshape(M, T).T)  # [100, 512]
    xhl = np.ascontiguousarray(np.stack(_split16(xt)))        # [2, 100, 512]
    whl = np.stack(_split16(Wc))                              # [2, 100, padded]
    in_maps = [
        {
            "XHL": xhl,
            "WHL": np.ascontiguousarray(whl[:, :, i * nsh : (i + 1) * nsh]),
        }
        for i in range(NCORES)
    ]
    nc = _build(nsh)

    def assemble(results):
        compact = np.concatenate([results[i]["OUT"] for i in range(NCORES)], axis=1)
        full = np.zeros((M, NDT), dtype=np.float32)
        full[:, nz] = compact[:, : len(nz)]
        return full.reshape(B, C, N_SMP, D_PROP, T)

    return nc, in_maps, assemble


def kernel(X, smp_weight):
    nc, in_maps, assemble = prepare_run(X, smp_weight)
    res = bass_utils.run_bass_kernel_spmd(nc, in_maps, core_ids=list(range(NCORES)))
    return assemble(res.results)



# revision 5
# speedup vs baseline: 5.8027x; 5.8027x over previous
"""Trainium2 Bass kernel for nn_BMSampling: out = X.reshape(B*C, T) @ smp_weight.

Strategy:
- smp_weight columns are <=2-tap interpolation stencils: 55.6% are entirely
  zero (output exactly 0.0) and the 142144 nonzero columns repeat the same
  (row, value-pair) stencil over and over -- only 6039 BIT-DISTINCT columns
  exist. The kernel dedups columns at runtime (generic for any weight: a
  fast <=2-adjacent-nonzero signature path with a full-column-bytes
  fallback), computes OUT_u = X @ W_unique on device, and expands with a
  single host-side gather (full[:, col] = OUT_u[:, inv[col]]; zero columns
  point at an all-zero padding column). This is the same class of host
  index bookkeeping as the zero-column scatter, extended to duplicates.
- Tensor-parallel over the ~6k unique columns: 8 cores x nsh (~760) each.
  Each core computes OUT[512, nsh] = XT[100,512].T @ Wu[100,nsh].
- fp32 matmul runs at 1/4 rate on trn2, so X and W are split into fp16
  hi/lo pairs on host and OUT = Xh@Wh + Xl@Wh + Xh@Wl accumulated in fp32
  PSUM (dropped Xl@Wl ~ 2^-22; end-to-end error ~1e-7 of scale).
- X loads ride the SP HWDGE ring while W strips ride the ACT ring (overlap
  at startup); output stores alternate across both rings.
"""

from contextlib import ExitStack

import numpy as np

import concourse.bacc as bacc
import concourse.mybir as mybir
import concourse.tile as tile
from concourse import bass_utils

B, C, T = 4, 128, 100
N_SMP, D_PROP = 32, 100
M = B * C                     # 512 matmul rows
NDT = N_SMP * D_PROP * T      # 320000 output columns
NCORES = 8
GRANULE = 8 * NCORES          # unique col count padded to this

K = T                         # 100 contraction dim (on SBUF partitions)
N_INNER = 512                 # matmul free dim cap (one PSUM bank of f32)
F32 = mybir.dt.float32
F16 = mybir.dt.float16

_PROGRAMS = {}


def _build(nsh):
    """Per-core program computing OUT[512, nsh] = XT.T @ W[100, nsh]."""
    if nsh in _PROGRAMS:
        return _PROGRAMS[nsh]

    strips = []
    s = 0
    while s < nsh:
        w = min(N_INNER, nsh - s)
        strips.append((s, w))
        s += w

    nc = bacc.Bacc("TRN2", debug=False)
    xhl = nc.dram_tensor("XHL", [K, 2, M], F16, kind="ExternalInput").ap()
    whl = nc.dram_tensor("WHL", [K, 2, nsh], F16, kind="ExternalInput").ap()
    out = nc.dram_tensor("OUT", [M, nsh], F32, kind="ExternalOutput").ap()

    with tile.TileContext(nc) as tc, ExitStack() as ctx:
        xpool = ctx.enter_context(tc.tile_pool(name="x", bufs=1))
        wpool = ctx.enter_context(tc.tile_pool(name="w", bufs=1))
        opool = ctx.enter_context(tc.tile_pool(name="o", bufs=4))
        pspool = ctx.enter_context(tc.tile_pool(name="ps", bufs=4, space="PSUM"))

        # X on the SP ring, W strips on the ACT ring: both stream in parallel
        # so the first matmul starts as soon as X + strip 0 land.
        x_sb = xpool.tile([K, 2, M], F16)
        nc.sync.dma_start(out=x_sb[:], in_=xhl[:])
        xh_sb = x_sb[:, 0]
        xl_sb = x_sb[:, 1]

        w_sb = wpool.tile([K, 2, nsh], F16)
        for s0, w in strips:
            nc.scalar.dma_start(
                out=w_sb[:, :, s0 : s0 + w], in_=whl[:, :, s0 : s0 + w]
            )

        store_engines = [nc.sync, nc.scalar]
        for m in range(M // 128):
            msl = slice(m * 128, (m + 1) * 128)
            o_sb = opool.tile([128, nsh], F32, tag="o_sb")
            for si, (s0, w) in enumerate(strips):
                ps = pspool.tile([128, N_INNER], F32)  # one PSUM bank
                dst = ps[:, :w]
                wh = w_sb[:, 0, s0 : s0 + w]
                wl = w_sb[:, 1, s0 : s0 + w]
                nc.tensor.matmul(dst, xh_sb[:, msl], wh, start=True, stop=False)
                nc.tensor.matmul(dst, xl_sb[:, msl], wh, start=False, stop=False)
                nc.tensor.matmul(dst, xh_sb[:, msl], wl, start=False, stop=True)
                if si % 2 == 0:  # GPSIMD can't read PSUM: DVE + ACT only
                    nc.vector.tensor_copy(out=o_sb[:, s0 : s0 + w], in_=dst)
                else:
                    nc.scalar.copy(out=o_sb[:, s0 : s0 + w], in_=dst)
            store_engines[m % 2].dma_start(out=out[msl, :], in_=o_sb[:])

    nc.compile()
    _PROGRAMS[nsh] = nc
    return nc


def _split16(a):
    hi = a.astype(np.float16)
    lo = (a - hi.astype(np.float32)).astype(np.float16)
    return hi, lo


def _dedup_columns(Wfull):
    """Returns (nz, first, inv): nonzero col indices, first-occurrence index
    into cols[nz] of each unique column, and inverse map len(nz)->len(first).
    Bit-exact dedup; fast path for <=2-adjacent-nonzero stencil columns."""
    cols = Wfull.T  # [NDT, K] view
    nz = np.flatnonzero((Wfull != 0).any(axis=0))
    colsnz = np.ascontiguousarray(cols[nz])
    n, k = colsnz.shape

    ar = np.arange(n)
    nzmask = colsnz != 0
    idx = np.argmax(nzmask, axis=1)
    nxt = np.minimum(idx + 1, k - 1)
    v1 = colsnz[ar, idx]
    v2 = np.where(nxt > idx, colsnz[ar, nxt], np.float32(0.0))
    nnz = nzmask.sum(axis=1)
    if np.all(nnz == 1 + (v2 != 0)):
        sig = np.empty(n, dtype=[("r", "<i4"), ("a", "<i4"), ("b", "<i4")])
        sig["r"] = idx
        sig["a"] = v1.view(np.int32)
        sig["b"] = v2.astype(np.float32).view(np.int32)
        _, first, inv = np.unique(sig, return_index=True, return_inverse=True)
    else:
        v = colsnz.view([("", np.void, k * 4)]).ravel()
        _, first, inv = np.unique(v, return_index=True, return_inverse=True)
    return nz, colsnz[first], inv


def prepare_run(X, smp_weight):
    """Returns (nc, in_maps, assemble) where assemble(results)->full output."""
    X = np.ascontiguousarray(np.asarray(X, dtype=np.float32))
    Wfull = np.asarray(smp_weight, dtype=np.float32)

    nz, ucols, inv = _dedup_columns(Wfull)
    U = len(ucols)
    # +1 guarantees at least one all-zero padding column for the gather below.
    padded = (U + 1 + GRANULE - 1) // GRANULE * GRANULE
    nsh = padded // NCORES
    Wu = np.zeros((K, padded), dtype=np.float32)
    Wu[:, :U] = ucols.T

    # zero output columns point at padding column U (exactly 0.0 on device)
    colmap = np.full(NDT, U, dtype=np.int32)
    colmap[nz] = inv

    xt = np.ascontiguousarray(X.reshape(M, T).T)              # [100, 512]
    xhl = np.ascontiguousarray(np.stack(_split16(xt), axis=1))  # [100, 2, 512]
    whl = np.stack(_split16(Wu), axis=1)                      # [100, 2, padded]
    in_maps = [
        {
            "XHL": xhl,
            "WHL": np.ascontiguousarray(whl[:, :, i * nsh : (i + 1) * nsh]),
        }
        for i in range(NCORES)
    ]
    nc = _build(nsh)

    def assemble(results):
        compact = np.concatenate([results[i]["OUT"] for i in range(NCORES)], axis=1)
        full = np.empty((M, NDT), dtype=np.float32)
        for i in range(M):  # per-row 1D takes: source row stays cache-resident
            np.take(compact[i], colmap, out=full[i])
        return full.reshape(B, C, N_SMP, D_PROP, T)

    return nc, in_maps, assemble


def kernel(X, smp_weight):
    nc, in_maps, assemble = prepare_run(X, smp_weight)
    res = bass_utils.run_bass_kernel_spmd(nc, in_maps, core_ids=list(range(NCORES)))
    return assemble(res.results)
